# revision 19
# baseline (speedup 1.0000x reference)
"""Trainium2 Bass kernel for nn_Attention_85169201480311.

Dense transformer block: 3x (conv3x3 -> GroupNorm(1) -> exact GELU) projections,
8-head attention over 1024 tokens with relative-position bias, 1x1 out-conv.

Sharding: data-parallel over batch (8 samples -> 8 cores), params replicated.

Per-core program:
 - x arrives host-pre-padded [128, 2, 34, 34] so the input DMA is one
   contiguous burst; a short warm-up matmul loop hides it and spins the PE
   clock monitor up to full rate.
 - conv3x3: per [P,512] output group, 18 taps x 2 K-split matmuls (2x64-row
   groups, so each row-group's LDWEIGHTS overlaps the other's streaming
   matmul); the halves are summed during eviction (scalar copy + DVE add)
   into an SBUF staging tile, then GroupNorm-affine + exact-GELU ACT.
   rstd comes from a fused 2-step Newton iteration on the DVE (no Sqrt
   table load; only gelu and exp table sets are ever loaded).
 - After the convs, a dummy-matmul heater bridges the PE-idle window of the
   v-eviction chain so the clock monitor stays at 8/8 into the attention
   stream; v is transposed into [v | ones] form via PE transposes.
 - scores^T tiles [128,1024] fp32 in a 3-slot PSUM ring; head pairs run
   row-packed. The softmax numerator exp(s*SCALE)*exp(bias) is computed one
   of three ways, statically assigned per tile to balance engines:
     PE path: bias/SCALE added into the scores PSUM by an identity-matmul
       accumulation, then a plain ACT exp (no multiply);
     scalar path: ACT exp then in-place DVE multiply by exp(bias) (bf16);
     fast path: one DVE affine_then_add computing the Schraudolph bit-trick
       exp fused with the bias in log2 domain, written as int16 directly
       into the bf16 attnT tile (bitcast); ~2% max relative error, which
       softmax normalization mostly cancels.
 - attn@v: per (pair, nj-half) [*,512] PSUM accumulators, lhsT = [v | ones]
   so rows 32/96 carry the softmax denominators; av units lag the score
   stream by 2 slots. At pair end the used rows are staged to SBUF by
   scalar copies (freeing the PSUM for the next pair), reciprocals via a
   [128,8] DMA respread, broadcast back by a stride-0 DMA, normalize on DVE
   into bf16 attn_out.
 - 1x1 out-conv (bf16) accumulates from attn_out in ring slots,
   Identity+bias ACT eviction, per-chunk output DMA.

Measured: 200.6 us HW exec (baseline 239.1 us), rel err 5.9e-3 (gate 2e-2).
"""
import sys
for p in ('/opt/trn_rl_repo', '/root/.axon_site/_ro/trn_rl_repo'):
    if p not in sys.path:
        sys.path.insert(0, p)

import numpy as np
import ml_dtypes

import concourse.bass as bass
import concourse.tile as tile
from concourse import mybir, bacc, bass_isa
from concourse import bass_utils
from concourse.masks import make_identity

F32 = mybir.dt.float32
BF16 = mybir.dt.bfloat16
AF = mybir.ActivationFunctionType
ALU = mybir.AluOpType

IH = IW = 32
N = IH * IW          # 1024 tokens
C = 256
HEADS = 8
DH = 32
SCALE = 32 ** -0.5
EPS = 1e-6
B = 8
P = 128
NCHUNK = C // P      # 2 channel chunks

N_WARM = 12

# fast-exp (Schraudolph in bf16 bit-space): exp(SCALE*s)*eb =
# bitcast_bf16(int16(s*A + B + L)) with A = SCALE*log2(e)*2^7,
# L = bias*log2(e)*2^7 (small, bf16-safe). Single DVE op per tile.
FAST_A = float(SCALE * 1.4426950408889634 * 128)
FAST_B = float((127 << 7) - 5.59)
I16 = mybir.dt.int16
I8 = mybir.dt.int8

# per-pair local tile indices (0..15):
FAST_LOCAL = {1, 3, 5, 7, 9, 11, 13}        # Schraudolph path (one DVE op)
# tiles whose bias is added into the scores PSUM by an identity-matmul on the
# PE (bias pre-divided by SCALE host-side); their ACT exp needs no multiply.
PE_LOCAL = {0, 2, 4, 6, 8, 10}

_cache = {}


def _rel_index():
    coords = np.stack(np.meshgrid(np.arange(IH), np.arange(IW), indexing='ij')).reshape(2, -1)
    rel = coords[:, :, None] - coords[:, None, :]
    rel[0] += IH - 1
    rel[1] += IW - 1
    rel[0] *= 2 * IW - 1
    return rel.sum(0)  # [n, m] int


def build_nc():
    nc = bacc.Bacc('TRN2', target_bir_lowering=False)

    xpad_d = nc.dram_tensor("xpad", [P, NCHUNK, IH + 2, IW + 2], BF16,
                            kind="ExternalInput")
    w_d = {}
    for nm in ("wq", "wk", "wv"):
        w_d[nm] = nc.dram_tensor(nm, [P, NCHUNK, 9, C], BF16, kind="ExternalInput")
    wout_d = nc.dram_tensor("wout", [P, NCHUNK, C], BF16, kind="ExternalInput")
    vecs_d = nc.dram_tensor("vecs", [P, 14], F32, kind="ExternalInput")
    eb_d = nc.dram_tensor("ebias", [HEADS, 8, P, N], BF16, kind="ExternalInput")
    ebl8_d = nc.dram_tensor("ebias8", [HEADS, 8, P, N], I8, kind="ExternalInput")
    eba_d = nc.dram_tensor("ebiasadd", [HEADS, 8, P, N], BF16, kind="ExternalInput")
    out_d = nc.dram_tensor("out", [P, NCHUNK, N], F32, kind="ExternalOutput")

    # vecs cols: gq0,gq1,bq0,bq1, gk0,gk1,bk0,bk1, gv0,gv1,bv0,bv1, bout0,bout1
    VGCOL = {"wq": 0, "wk": 4, "wv": 8}

    with tile.TileContext(nc) as tc:
        with tc.tile_pool(name="const", bufs=1) as const, \
             tc.tile_pool(name="proj", bufs=1) as proj, \
             tc.tile_pool(name="stats", bufs=2) as stats_p, \
             tc.tile_pool(name="attn", bufs=1) as attn_p:

            xpad = const.tile([P, NCHUNK, IH + 2, IW + 2], BF16)
            vecs = const.tile([P, 14], F32)
            wout_sb = const.tile([P, NCHUNK, C], BF16)
            ident = const.tile([P, P], BF16)

            q_sb = proj.tile([P, NCHUNK, N], BF16)
            k_sb = proj.tile([P, NCHUNK, N], BF16)
            v_sb = proj.tile([P, NCHUNK, N], BF16)
            v_aug = proj.tile([P, 8, 8, 34], BF16)   # [*, i, h, 0:32 v | 32 ones]
            attn_out = attn_p.tile([P, NCHUNK, N], BF16)
            out_sb = attn_p.tile([P, NCHUNK, N], F32)

            w_sb = {}

            # ---------- GroupNorm stats chain (shared by q,k,v) ----------
            def gn_chain(nm, st_t):
                mv = [stats_p.tile([P, 2], F32, name=f"mv_{nm}_{m}", tag="mv")
                      for m in range(2)]
                for m in range(2):
                    nc.vector.bn_aggr(out=mv[m][:], in_=st_t[m][:])
                prep = stats_p.tile([P, 4], F32, name=f"prep_{nm}", tag="prep")
                for m in range(2):
                    nc.vector.tensor_copy(out=prep[:, 2 * m:2 * m + 1], in_=mv[m][:, 0:1])
                    sq = stats_p.tile([P, 1], F32, name=f"sq_{nm}_{m}", tag="sq")
                    nc.vector.tensor_mul(out=sq[:], in0=mv[m][:, 0:1], in1=mv[m][:, 0:1])
                    nc.vector.tensor_add(out=prep[:, 2 * m + 1:2 * m + 2],
                                         in0=mv[m][:, 1:2], in1=sq[:])
                # cross-partition reduce WITHOUT gpsimd (its queue is reserved
                # for bias-tile DMAs): PE ones-matmul sums prep over
                # partitions into a [4,1] PSUM column, tiny copy + respread +
                # stride-0 broadcast bring it back to [P,4].
                gnp = psc.tile([4, 1], F32, name=f"gnp_{nm}", tag="aux",
                               bufs=1)
                nc.tensor.matmul(gnp[:], prep[:], ones_sb[:, 0:1],
                                 start=True, stop=True)
                srow = stats_p.tile([4, 1], F32, name=f"srow_{nm}", tag="srow")
                nc.scalar.copy(out=srow[:], in_=gnp[:])
                rrow = stats_p.tile([1, 4], F32, name=f"rrow_{nm}", tag="rrow")
                nc.sync.dma_start(out=rrow[:], in_=srow[:])
                red = stats_p.tile([P, 4], F32, name=f"red_{nm}", tag="red")
                ra = rrow[:]
                rsrc = bass.AP(tensor=ra.tensor, offset=ra.offset,
                               ap=[list(ra.ap[0]), [0, P]]
                               + [list(d) for d in ra.ap[1:]])
                nc.sync.dma_start(out=red[:], in_=rsrc)
                mt = stats_p.tile([P, 4], F32, name=f"mt_{nm}", tag="mt")
                nc.vector.tensor_add(out=mt[:, 0:1], in0=red[:, 0:1], in1=red[:, 2:3])
                nc.scalar.mul(out=mt[:, 0:1], in_=mt[:, 0:1], mul=1.0 / C)
                nc.vector.tensor_add(out=mt[:, 1:2], in0=red[:, 1:2], in1=red[:, 3:4])
                nc.scalar.mul(out=mt[:, 1:2], in_=mt[:, 1:2], mul=1.0 / C)
                nc.vector.tensor_mul(out=mt[:, 2:3], in0=mt[:, 0:1], in1=mt[:, 0:1])
                nc.vector.tensor_sub(out=mt[:, 1:2], in0=mt[:, 1:2], in1=mt[:, 2:3])
                # rstd = 1/sqrt(var+eps): reciprocal + Newton (no Sqrt table)
                ve = stats_p.tile([P, 1], F32, name=f"ve_{nm}", tag="ve")
                nc.vector.tensor_scalar_add(out=ve[:], in0=mt[:, 1:2], scalar1=EPS)
                y = stats_p.tile([P, 1], F32, name=f"y_{nm}", tag="y")
                nc.vector.reciprocal(out=y[:], in_=ve[:])
                nc.vector.tensor_scalar(out=y[:], in0=y[:], scalar1=0.5, scalar2=0.5,
                                        op0=ALU.mult, op1=ALU.add)
                t1 = stats_p.tile([P, 1], F32, name=f"t1_{nm}", tag="t1")
                for _ in range(2):
                    nc.vector.tensor_mul(out=t1[:], in0=y[:], in1=y[:])
                    nc.vector.scalar_tensor_tensor(out=t1[:], in0=t1[:], scalar=-0.5,
                                                   in1=ve[:], op0=ALU.mult,
                                                   op1=ALU.mult)
                    nc.vector.scalar_tensor_tensor(out=y[:], in0=t1[:], scalar=1.5,
                                                   in1=y[:], op0=ALU.add,
                                                   op1=ALU.mult)
                gc = VGCOL[nm]
                sc = stats_p.tile([P, 4], F32, name=f"sc_{nm}", tag="sc")
                for m in range(2):
                    nc.vector.tensor_mul(out=sc[:, m:m + 1],
                                         in0=vecs[:, gc + m:gc + m + 1], in1=y[:])
                    nc.vector.tensor_mul(out=sc[:, 2 + m:3 + m],
                                         in0=mt[:, 0:1], in1=sc[:, m:m + 1])
                    nc.vector.tensor_sub(out=sc[:, 2 + m:3 + m],
                                         in0=vecs[:, gc + 2 + m:gc + 3 + m],
                                         in1=sc[:, 2 + m:3 + m])
                return sc

            CONV_ITEMS = [(c, t) for c in range(NCHUNK) for t in range(9)]

            # ---------------- q,k conv (direct GELU eviction) ----------------
            with tc.tile_pool(name="wpool", bufs=1) as wpool, \
                 tc.tile_pool(name="psc", bufs=1, space="PSUM") as psc:
                warm_sb = wpool.tile([P, 512], BF16)
                nc.vector.memset(warm_sb[:], 0.0)
                ones_sb = wpool.tile([P, 1], F32, name="ones_sb")
                nc.vector.memset(ones_sb[:], 1.0)
                for nm in ("wq", "wk", "wv"):
                    w_sb[nm] = wpool.tile([P, NCHUNK, 9, C], BF16, name=f"sb_{nm}")
                nc.sync.dma_start(w_sb["wq"][:], w_d["wq"][:])
                nc.sync.dma_start(xpad[:], xpad_d[:])
                nc.sync.dma_start(vecs[:], vecs_d[:])
                heat_n = [0]

                def heater():
                    ps_w = psc.tile([P, 512], F32, name=f"heat_{heat_n[0]}",
                                    tag="cv", bufs=6)
                    heat_n[0] += 1
                    nc.tensor.matmul(ps_w[:], warm_sb[:, 0:P], warm_sb[:],
                                     start=True, stop=True)

                for _ in range(N_WARM):
                    heater()
                for nm in ("wk", "wv"):
                    nc.sync.dma_start(w_sb[nm][:], w_d[nm][:])
                nc.sync.dma_start(wout_sb[:], wout_d[:])
                make_identity(nc, ident[:])
                nc.vector.memset(v_aug[:, :, :, 32:33], 1.0)

                # conv order (wv, wq, wk): v ready early so its transposes
                # interleave under wk's matmul stream; each conv's GroupNorm
                # chain + GELU evictions are emitted lazily after the NEXT
                # conv's first matmul group so the in-order PE queue never
                # stalls on the stats chain.
                pending_gn = [None]

                def emit_gn_and_evict():
                    nm, dst, ps_t, st_t = pending_gn[0]
                    sc = gn_chain(nm, st_t)
                    for m in range(2):
                        for j in range(2):
                            nc.scalar.activation(
                                out=dst[:, m, 512 * j:512 * (j + 1)],
                                in_=ps_t[m * 2 + j][:],
                                func=AF.Gelu, scale=sc[:, m:m + 1],
                                bias=sc[:, 2 + m:3 + m])
                    pending_gn[0] = None

                trans_left = [(kk, ii) for kk in range(NCHUNK) for ii in range(8)]

                def transpose_unit():
                    if not trans_left:
                        return
                    kk, ii = trans_left.pop(0)
                    tag = "pvt" if (kk * 8 + ii) % 2 == 0 else "aux"
                    pvt = psc.tile([P, P], BF16, name=f"vt_{kk}_{ii}",
                                   tag=tag, bufs=1)
                    nc.tensor.transpose(
                        pvt[:], v_sb[:, kk, P * ii:P * (ii + 1)], ident[:])
                    nc.vector.tensor_copy(
                        out=v_aug[:, ii, 4 * kk:4 * kk + 4, 0:32], in_=pvt[:])

                for nm, dst in (("wv", v_sb), ("wq", q_sb), ("wk", k_sb)):
                    ps_t = []
                    st_t = [stats_p.tile([P, 2, 6], F32, name=f"st_{nm}_{m}", tag="st")
                            for m in range(2)]
                    for m in range(2):
                        for j in range(2):
                            ps = psc.tile([P, 512], F32, name=f"cv_{nm}_{m}_{j}",
                                          tag="cv", bufs=6)
                            for idx, (cc, t) in enumerate(CONV_ITEMS):
                                dy, dx = t // 3, t % 3
                                nc.tensor.matmul(
                                    ps[:],
                                    w_sb[nm][:, cc, t, m * P:(m + 1) * P],
                                    xpad[:, cc, 16 * j + dy:16 * j + dy + 16,
                                         dx:dx + 32],
                                    start=(idx == 0), stop=(idx == 17))
                            if (m, j) == (0, 0) and pending_gn[0] is not None:
                                emit_gn_and_evict()
                            if nm == "wq" and (m, j) != (0, 0):
                                for _ in range(3):
                                    transpose_unit()
                            elif nm == "wk":
                                for _ in range(2):
                                    transpose_unit()
                            nc.vector.bn_stats(out=st_t[m][:, j, :], in_=ps[:])
                            ps_t.append(ps)
                    if nm == "wk":
                        while trans_left:
                            transpose_unit()
                    pending_gn[0] = (nm, dst, ps_t, st_t)
                # wk's chain: bridge the stats-chain latency with heaters so
                # the PE-HAM clock stays at 8/8 into the attention stream
                for _ in range(4):
                    heater()
                emit_gn_and_evict()
                for _ in range(10):
                    heater()

            # trigger the exp ACT table load early (overlaps the conv tail)
            preheat = attn_p.tile([P, 1], BF16, name="preheat")
            nc.scalar.activation(out=preheat[:], in_=vecs[:, 0:1], func=AF.Exp,
                                 scale=1.0)

            # ---------------- attention ----------------
            from contextlib import ExitStack
            with ExitStack() as stk:
                ring_p = stk.enter_context(tc.tile_pool(name="ring_p", bufs=3,
                                                        space="PSUM"))
                av_pool = stk.enter_context(tc.tile_pool(name="av_p", bufs=2,
                                                         space="PSUM"))
                attnT_p = stk.enter_context(tc.tile_pool(name="attnT_p", bufs=4))
                ebias_p = stk.enter_context(tc.tile_pool(name="ebias_p", bufs=9))
                ebias8_p = stk.enter_context(tc.tile_pool(name="ebias8_p", bufs=12))
                rc_p = stk.enter_context(tc.tile_pool(name="rc", bufs=2))

                attnTs = {}
                eb_tiles = {}
                av_tiles = {}
                dma_rr = [0]

                def issue_eb(p, i):
                    for h in (2 * p, 2 * p + 1):
                        l = (i % 8) * 2 + (h & 1)
                        if l in FAST_LOCAL:
                            t = ebias8_p.tile([P, N], I8, name=f"eb_{h}_{i}",
                                              tag="eb8")
                            src = ebl8_d
                        else:
                            t = ebias_p.tile([P, N], BF16, name=f"eb_{h}_{i}",
                                             tag="eb")
                            src = eba_d if l in PE_LOCAL else eb_d
                        eng = nc.gpsimd if (dma_rr[0] % 2) else nc.sync
                        dma_rr[0] += 1
                        eng.dma_start(t[:], src[h, i])
                        eb_tiles[(h, i)] = t

                def scores_slot(p, i):
                    h0, h1 = 2 * p, 2 * p + 1
                    rts = {}
                    for h in (h0, h1):
                        rts[h] = ring_p.tile([P, N], F32, name=f"s_{h}_{i}", tag="ring")
                    for nj in range(2):
                        for h in (h0, h1):
                            g, r = h // 4, h % 4
                            l = (i % 8) * 2 + (h & 1)
                            nc.tensor.matmul(
                                rts[h][:, 512 * nj:512 * (nj + 1)],
                                k_sb[32 * r:32 * r + 32, g, P * i:P * (i + 1)],
                                q_sb[32 * r:32 * r + 32, g, 512 * nj:512 * (nj + 1)],
                                start=True, stop=(l not in PE_LOCAL),
                                tile_position=(32 * r, 0))
                    # PE-side additive bias: psum += I.T @ (bias/SCALE)
                    for h in (h0, h1):
                        l = (i % 8) * 2 + (h & 1)
                        if l in PE_LOCAL:
                            eb_t = eb_tiles[(h, i)]
                            for nj in range(2):
                                nc.tensor.matmul(
                                    rts[h][:, 512 * nj:512 * (nj + 1)],
                                    ident[:],
                                    eb_t[:, 512 * nj:512 * (nj + 1)],
                                    start=False, stop=True)
                    return rts

                def exp_tile(p, i, h, rt):
                    l = (i % 8) * 2 + (h & 1)
                    eb_t = eb_tiles.pop((h, i))
                    dst = attnTs[h][:, i, :]
                    if l in FAST_LOCAL:
                        nc.vector.affine_then_add(out=dst.bitcast(I16),
                                                  in0=rt[:], in1=eb_t[:],
                                                  scale=FAST_A, bias=FAST_B)
                    elif l in PE_LOCAL:
                        nc.scalar.activation(out=dst, in_=rt[:], func=AF.Exp,
                                             scale=SCALE)
                    else:
                        nc.scalar.activation(out=dst, in_=rt[:], func=AF.Exp,
                                             scale=SCALE)
                        nc.vector.tensor_mul(out=dst, in0=dst, in1=eb_t[:])

                def av_unit(p, i):
                    h0, h1 = 2 * p, 2 * p + 1
                    if i == 0:
                        for nj in range(2):
                            av_tiles[nj] = av_pool.tile(
                                [P, 512], F32, name=f"av_{p}_{nj}", tag="av")
                    for nj in range(2):
                        for h in (h0, h1):
                            rv = h % 2
                            nc.tensor.matmul(
                                av_tiles[nj][64 * rv:64 * rv + 33, :],
                                v_aug[:, i, h, 0:33],
                                attnTs[h][:, i, 512 * nj:512 * (nj + 1)],
                                start=(i == 0), stop=(i == 7),
                                tile_position=(0, 64 * rv))
                    if i == 7:
                        avs = rc_p.tile([P, 2, 512], F32, name=f"avs_{p}", tag="avs")
                        for nj in range(2):
                            eng = nc.scalar if nj == 0 else nc.vector
                            for rv in range(2):
                                if nj == 0:
                                    eng.copy(
                                        out=avs[64 * rv:64 * rv + 33, nj, :],
                                        in_=av_tiles[nj][64 * rv:64 * rv + 33, :])
                                else:
                                    eng.tensor_copy(
                                        out=avs[64 * rv:64 * rv + 33, nj, :],
                                        in_=av_tiles[nj][64 * rv:64 * rv + 33, :])
                        pending.append([None, norm_steps(p, avs)])

                def norm_steps(p, avs):
                    grp = p // 2
                    h0, h1 = 2 * p, 2 * p + 1
                    sp = rc_p.tile([P, 16], F32, name=f"sp_{p}", tag="sp")
                    for nj in range(2):
                        for rv in range(2):
                            c = 4 * (2 * nj + rv)
                            nc.sync.dma_start(out=sp[:, c:c + 4],
                                              in_=avs[64 * rv + 32:64 * rv + 33,
                                                      nj, :])
                    yield
                    nc.vector.reciprocal(out=sp[:], in_=sp[:])
                    yield
                    rrow = rc_p.tile([64, 2, 512], F32, name=f"rr_{p}", tag="rr")
                    for nj in range(2):
                        for rv in range(2):
                            c = 4 * (2 * nj + rv)
                            nc.sync.dma_start(out=rrow[32 * rv:32 * rv + 1, nj, :],
                                              in_=sp[:, c:c + 4])
                    yield
                    rcp_bc = rc_p.tile([P, 2, 512], F32, name=f"rb_{p}", tag="rb")
                    for nj in range(2):
                        for rv in range(2):
                            rowap = rrow[32 * rv:32 * rv + 1, nj, :]
                            src = bass.AP(tensor=rowap.tensor, offset=rowap.offset,
                                          ap=[list(rowap.ap[0]), [0, 32]]
                                          + [list(d) for d in rowap.ap[1:]])
                            nc.sync.dma_start(out=rcp_bc[64 * rv:64 * rv + 32, nj, :],
                                              in_=src)
                    yield
                    for h in (h0, h1):
                        r, rv = h % 4, h % 2
                        nc.vector.tensor_mul(
                            out=attn_out[32 * r:32 * r + 32, grp, :],
                            in0=avs[64 * rv:64 * rv + 32, :, :],
                            in1=rcp_bc[64 * rv:64 * rv + 32, :, :])
                    yield

                def outconv(nj):
                    ps_o = ring_p.tile([P, N], F32, name=f"o_{nj}", tag="ring")
                    for m in range(2):
                        for kk in range(NCHUNK):
                            nc.tensor.matmul(
                                ps_o[:, 512 * m:512 * (m + 1)],
                                wout_sb[:, kk, m * P:(m + 1) * P],
                                attn_out[:, kk, 512 * nj:512 * (nj + 1)],
                                start=(kk == 0), stop=(kk == NCHUNK - 1))
                        nc.scalar.activation(
                            out=out_sb[:, m, 512 * nj:512 * (nj + 1)],
                            in_=ps_o[:, 512 * m:512 * (m + 1)],
                            func=AF.Identity,
                            bias=vecs[:, 12 + m:13 + m], scale=1.0)
                        nc.sync.dma_start(out_d[:, m, 512 * nj:512 * (nj + 1)],
                                          out_sb[:, m, 512 * nj:512 * (nj + 1)])

                # ---- main attention loop ----
                av_next = [0]
                pending = []

                def emit_av(upto, cap=4):
                    n = 0
                    while av_next[0] <= upto and n < cap:
                        cs = av_next[0]
                        av_unit(cs // 8, cs % 8)
                        av_next[0] += 1
                        n += 1

                def step_pending():
                    done = []
                    for item in pending:
                        try:
                            next(item[1])
                        except StopIteration:
                            done.append(item)
                    for item in done:
                        pending.remove(item)

                for s0 in range(6):
                    issue_eb(s0 // 8, s0 % 8)
                for s in range(32):
                    p, i = s // 8, s % 8
                    if i == 0:
                        for h in (2 * p, 2 * p + 1):
                            attnTs[h] = attnT_p.tile([P, 8, N], BF16,
                                                     name=f"attnT_{h}", tag="attnT")
                    nx = s + 6
                    if nx < 32:
                        issue_eb(nx // 8, nx % 8)
                    rts = scores_slot(p, i)
                    if s >= 2:
                        emit_av(s - 2)
                    for h in (2 * p, 2 * p + 1):
                        exp_tile(p, i, h, rts[h])
                    step_pending()
                emit_av(31, cap=32)
                while pending:
                    step_pending()
                outconv(0)
                outconv(1)

    nc.compile()
    return nc


def _prep_shared(Wq, gq, bq, Wk, gk, bk, Wv, gv, bv, bias_table, Wout, bout):
    def wt(W):
        return np.ascontiguousarray(
            W.astype(np.float32).transpose(1, 2, 3, 0).reshape(NCHUNK, P, 9, C)
            .transpose(1, 0, 2, 3)).astype(ml_dtypes.bfloat16)
    vecs = np.zeros((P, 14), np.float32)
    for col, v in ((0, gq), (2, bq), (4, gk), (6, bk), (8, gv), (10, bv), (12, bout)):
        vecs[:, col] = v[:P]
        vecs[:, col + 1] = v[P:]
    wout = np.ascontiguousarray(Wout[:, :, 0, 0].T.reshape(NCHUNK, P, C)
                                .transpose(1, 0, 2)).astype(ml_dtypes.bfloat16)
    idx = _rel_index()                     # [n, m]
    bt = bias_table.astype(np.float32)
    eb = np.exp(bt)[idx]                   # [n, m, H]
    ebT = eb.transpose(2, 1, 0)            # [H, m, n]
    ebias = np.ascontiguousarray(ebT.reshape(HEADS, 8, P, N).astype(ml_dtypes.bfloat16))
    ebl = np.clip(np.rint(bt * np.float32(1.4426950408889634 * 128)),
                  -127, 127).astype(np.int8)[idx]  # [n, m, H]
    eblT = ebl.transpose(2, 1, 0)
    ebias8 = np.ascontiguousarray(eblT.reshape(HEADS, 8, P, N))
    eba = (bt / np.float32(SCALE))[idx]
    ebaT = eba.transpose(2, 1, 0)
    ebiasadd = np.ascontiguousarray(
        ebaT.reshape(HEADS, 8, P, N).astype(ml_dtypes.bfloat16))
    return {"wq": wt(Wq), "wk": wt(Wk), "wv": wt(Wv), "vecs": vecs,
            "wout": wout, "ebias": ebias, "ebias8": ebias8,
            "ebiasadd": ebiasadd}


def kernel(x, Wq, gq, bq, Wk, gk, bk, Wv, gv, bv, bias_table, Wout, bout):
    x = np.asarray(x, np.float32)
    if "nc" not in _cache:
        _cache["nc"] = build_nc()
    nc = _cache["nc"]
    shared = _prep_shared(np.asarray(Wq), np.asarray(gq), np.asarray(bq),
                          np.asarray(Wk), np.asarray(gk), np.asarray(bk),
                          np.asarray(Wv), np.asarray(gv), np.asarray(bv),
                          np.asarray(bias_table), np.asarray(Wout), np.asarray(bout))
    in_maps = []
    for b in range(B):
        m = dict(shared)
        xb = x[b].reshape(NCHUNK, P, IH, IW).transpose(1, 0, 2, 3)
        xp = np.zeros((P, NCHUNK, IH + 2, IW + 2), np.float32)
        xp[:, :, 1:IH + 1, 1:IW + 1] = xb
        m["xpad"] = np.ascontiguousarray(xp.astype(ml_dtypes.bfloat16))
        in_maps.append(m)
    _cache["last_in_maps"] = in_maps
    res = bass_utils.run_bass_kernel_spmd(nc, in_maps, core_ids=list(range(B)))
    out = np.stack([r["out"] for r in res.results])          # [B, 128, 2, 1024]
    out = out.transpose(0, 2, 1, 3).reshape(B, C, IH, IW)
    return np.ascontiguousarray(out.astype(np.float32))


if __name__ == "__main__":
    rng = np.random.default_rng(0)
    inputs = {
        'x': rng.standard_normal((B, C, IH, IW), dtype=np.float32),
        'Wq': (rng.standard_normal((C, C, 3, 3)) * 0.02).astype(np.float32),
        'gq': np.ones(C, np.float32), 'bq': np.zeros(C, np.float32),
        'Wk': (rng.standard_normal((C, C, 3, 3)) * 0.02).astype(np.float32),
        'gk': np.ones(C, np.float32), 'bk': np.zeros(C, np.float32),
        'Wv': (rng.standard_normal((C, C, 3, 3)) * 0.02).astype(np.float32),
        'gv': np.ones(C, np.float32), 'bv': np.zeros(C, np.float32),
        'bias_table': (rng.standard_normal(((2 * IH - 1) * (2 * IW - 1), HEADS)) * 0.02).astype(np.float32),
        'Wout': (rng.standard_normal((C, C, 1, 1)) * 0.02).astype(np.float32),
        'bout': np.zeros(C, np.float32),
    }
    out = kernel(**inputs)
    print("out", out.shape, out.dtype, np.abs(out).max())

